# revision 1
# baseline (speedup 1.0000x reference)
"""Trainium2 Bass kernel for the AutoregressiveSplineDeep flow.

Computes 4 steps of a MADE-conditioned monotonic linear-rational-spline flow
over N=131072 2-d samples, data-parallel over 8 NeuronCores.

Structure per core (NS = 16384 samples):
  - dim-0 spline params are constants (MADE mask zeroes that path), so the
    z0 chain is 4 applications of one constant spline: evaluated with a
    per-sample bin search + gpsimd indirect-copy gather from constant tables.
  - dim-1 params come from the masked MLP (256-wide, bf16 matmuls on PE in
    channel-major layout); layer-3 is emitted sample-major (lhsT = h2 chunk)
    so spline tables build directly in samples-on-partitions layout.
  - softmax/cumsum of spline widths/heights via ACT exp + one masked
    tensor_tensor_scan; bin search via broadcast compares; all 7 per-sample
    table gathers in one gpsimd indirect_copy; the rational spline itself is
    evaluated on [128, NS/128] fp32 tiles.
"""

import sys

sys.path.insert(0, "/opt/trn_rl_repo")

import numpy as np
import ml_dtypes

INPUT_DIM = 2
COUNT_BINS = 16
BOUND = 5.0
FLOW_LENGTH = 4
HIDDEN = 256
MIN_BIN = 1e-3
MIN_DERIV = 1e-3
MIN_LAMBDA = 0.025
N_FULL = 131072
N_CORES = 8
NS = N_FULL // N_CORES  # 16384 per core

LEFT, RIGHT = -BOUND, BOUND
SCALE10 = RIGHT - LEFT  # 10
CFREE = 1.0 - MIN_BIN * COUNT_BINS  # 0.984
A10C = SCALE10 * CFREE  # 9.84


def _np_const_spline_tables(b3_even):
    """Mirror the reference's dim-0 (constant) spline tables in float64."""
    r = b3_even.astype(np.float64)
    w0, h0, d0, l0 = r[0:16], r[16:32], r[32:47], r[47:63]

    def soft(v):
        e = np.exp(v - v.max())
        return e / e.sum()

    widths = MIN_BIN + CFREE * soft(w0)
    cw = np.concatenate([[0.0], np.cumsum(widths)])
    cw = SCALE10 * cw + LEFT
    cw[0], cw[-1] = LEFT, RIGHT
    widths_f = np.diff(cw)
    heights = MIN_BIN + CFREE * soft(h0)
    ch = np.concatenate([[0.0], np.cumsum(heights)])
    ch = SCALE10 * ch + LEFT
    ch[0], ch[-1] = LEFT, RIGHT
    heights_f = np.diff(ch)
    deriv = np.concatenate([[1.0], MIN_DERIV + np.log1p(np.exp(d0)), [1.0]])
    lam = MIN_LAMBDA + (1.0 - 2.0 * MIN_LAMBDA) * (1.0 / (1.0 + np.exp(-l0)))
    tab = np.concatenate(
        [cw[0:16], widths_f, ch[0:16], heights_f, deriv[0:16], deriv[1:17], lam]
    ).astype(np.float32)  # [112]
    cmp0 = cw[1:16].astype(np.float32)  # [15]
    return tab, cmp0


def _build_program(ns):
    """Build the SPMD bass program for `ns` samples per core."""
    import concourse.bacc as bacc
    import concourse.tile as tile
    import concourse.mybir as mybir

    F32 = mybir.dt.float32
    BF16 = mybir.dt.bfloat16
    U8 = mybir.dt.uint8
    U16 = mybir.dt.uint16
    AF = mybir.ActivationFunctionType
    ALU = mybir.AluOpType

    CC = ns // 128          # columns of the samples-on-partitions tiles
    G = 16                  # sample-chunks per L3 psum tile (2 psum banks)
    BLK = 128 * G           # 1024 samples per spline stage-1 block
    NBLK = ns // BLK
    NF = ns // 512          # 512-sample F-tiles for L1/L2

    nc = bacc.Bacc("TRN2", target_bir_lowering=False, debug=False,
                   num_devices=N_CORES)

    def din(name, shape, dt=F32):
        return nc.dram_tensor(name, list(shape), dt, kind="ExternalInput").ap()

    def dout(name, shape, dt=F32):
        return nc.dram_tensor(name, list(shape), dt, kind="ExternalOutput").ap()

    # per-core data
    xsp = din("XSP", [128, CC, 2])
    x0rowb = din("X0ROWB", [1, ns], BF16)
    # weights / consts (replicated)
    w1cb = din("W1CB", [1, 256], BF16)
    b1t = din("B1T", [128, 2])
    w2tb = din("W2TB", [128, 2, 2, 128], BF16)
    b2t = din("B2T", [128, 2])
    w3tb = din("W3TB", [128, 2, 63], BF16)
    b3tab = din("B3TAB", [128, 63])
    tab0 = din("TAB0", [128, 112])
    cmp0 = din("CMP0", [128, 15])
    iota16 = din("IOTA16", [128, 16])
    b16t = din("B16T", [128, 15])
    m32 = din("M32", [128, G * 32])
    ident = din("IDENT", [128, 128])
    z0out = dout("Z0OUT", [FLOW_LENGTH, 128, CC])
    z1out = dout("Z1OUT", [FLOW_LENGTH, 128, CC])

    with tile.TileContext(nc) as tc:
        _emit(nc, tc, locals())
    nc.compile()
    return nc


def _emit(nc, tc, t):
    import concourse.mybir as mybir

    F32 = mybir.dt.float32
    BF16 = mybir.dt.bfloat16
    U8 = mybir.dt.uint8
    U16 = mybir.dt.uint16
    AF = mybir.ActivationFunctionType
    ALU = mybir.AluOpType

    CC, G, NBLK, NF = t["CC"], t["G"], t["NBLK"], t["NF"]
    ns = t["ns"]
    xsp, x0rowb = t["xsp"], t["x0rowb"]
    w1cb, b1t, w2tb, b2t, w3tb, b3tab = (
        t["w1cb"], t["b1t"], t["w2tb"], t["b2t"], t["w3tb"], t["b3tab"])
    tab0, cmp0, iota16, b16t, m32, ident = (
        t["tab0"], t["cmp0"], t["iota16"], t["b16t"], t["m32"], t["ident"])
    z0out, z1out = t["z0out"], t["z1out"]

    import contextlib
    ctx = contextlib.ExitStack()
    with ctx:
        consts = ctx.enter_context(tc.tile_pool(name="consts", bufs=1))
        zpool = ctx.enter_context(tc.tile_pool(name="z", bufs=1))
        rows = ctx.enter_context(tc.tile_pool(name="rows", bufs=2))
        pha = ctx.enter_context(tc.tile_pool(name="pha", bufs=1))
        s2pool = ctx.enter_context(tc.tile_pool(name="s2", bufs=2))
        mlp = ctx.enter_context(tc.tile_pool(name="mlp", bufs=4))
        mpsum = ctx.enter_context(tc.tile_pool(name="mp", bufs=1, space="PSUM"))
        s2psum = ctx.enter_context(tc.tile_pool(name="s2p", bufs=1, space="PSUM"))
        l3psum = ctx.enter_context(tc.tile_pool(name="l3p", bufs=2, space="PSUM"))
        blkpool = ctx.enter_context(tc.tile_pool(name="blk", bufs=2))

        # ---- load constants into SBUF
        def cload(ap, shape, dt=F32, tag=None):
            tl = consts.tile(shape, dt, tag=tag or ap.name)
            nc.sync.dma_start(tl[:], ap[:])
            return tl

        cW1 = cload(w1cb, [1, 256], BF16)
        cB1 = cload(b1t, [128, 2])
        cW2 = cload(w2tb, [128, 2, 2, 128], BF16)
        cB2 = cload(b2t, [128, 2])
        cW3 = cload(w3tb, [128, 2, 63], BF16)
        cB3 = cload(b3tab, [128, 63])
        cTAB0 = cload(tab0, [128, 112])
        cCMP0 = cload(cmp0, [128, 15])
        cIOTA = cload(iota16, [128, 16])
        cB16 = cload(b16t, [128, 15])
        cM32 = cload(m32, [128, G * 32])
        cIDENT = cload(ident, [128, 128])

        # ---------------- stage-2: rational spline on [128, CC] tiles -------
        def emit_stage2(q, binf, xc, xog, zdst, dim1):
            """q: dict of [128, CC] APs (xk, wd, yk, ht, d0, d1, lr);
            binf/xc/xog: [128, CC] APs; zdst: [128, CC] SBUF AP (output)."""
            P = s2pool

            def tl(tag):
                return P.tile([128, CC], F32, tag="s2_" + tag, name="s2_" + tag)

            def tt(out, a, b, op):
                nc.vector.tensor_tensor(out, a, b, op)

            if dim1:
                # DK = bin==0 ? 1 : MIN_DERIV + softplus(d0); same for DK1/15
                m0 = P.tile([128, CC], U8, tag="s2_m0")
                nc.vector.tensor_scalar(m0[:], binf, 0.5, None, ALU.is_lt)
                m15 = P.tile([128, CC], U8, tag="s2_m15")
                nc.vector.tensor_scalar(m15[:], binf, 14.5, None, ALU.is_gt)
                one = tl("one")
                nc.vector.memset(one[:], 1.0)
                dk = tl("dk")
                e1 = tl("e1")
                nc.scalar.activation(e1[:], q["d0"], AF.Exp)
                nc.vector.tensor_scalar(e1[:], e1[:], 1.0, None, ALU.add)
                nc.scalar.activation(e1[:], e1[:], AF.Ln)
                nc.vector.tensor_scalar(dk[:], e1[:], MIN_DERIV, None, ALU.add)
                nc.vector.copy_predicated(dk[:], m0[:], one[:])
                dk1 = tl("dk1")
                e2 = tl("e2")
                nc.scalar.activation(e2[:], q["d1"], AF.Exp)
                nc.vector.tensor_scalar(e2[:], e2[:], 1.0, None, ALU.add)
                nc.scalar.activation(e2[:], e2[:], AF.Ln)
                nc.vector.tensor_scalar(dk1[:], e2[:], MIN_DERIV, None, ALU.add)
                nc.vector.copy_predicated(dk1[:], m15[:], one[:])
                # lam = m + (1-2m) * sigmoid(lr); sigmoid = exp(-softplus(-x))
                lam = tl("lam")
                sg = tl("sg")
                nc.scalar.activation(sg[:], q["lr"], AF.Exp, scale=-1.0)
                nc.vector.tensor_scalar(sg[:], sg[:], 1.0, None, ALU.add)
                nc.scalar.activation(sg[:], sg[:], AF.Ln)
                nc.scalar.activation(sg[:], sg[:], AF.Exp, scale=-1.0)
                nc.vector.tensor_scalar(
                    lam[:], sg[:], 1.0 - 2.0 * MIN_LAMBDA, MIN_LAMBDA,
                    ALU.mult, ALU.add)
            else:
                dk, dk1 = tl("dk"), tl("dk1")
                nc.vector.tensor_copy(dk[:], q["d0"])
                nc.vector.tensor_copy(dk1[:], q["d1"])
                lam = tl("lam")
                nc.vector.tensor_copy(lam[:], q["lr"])

            # wb = sqrt(dk/dk1) = exp(0.5*(ln dk - ln dk1))
            lna = tl("lna")
            nc.scalar.activation(lna[:], dk[:], AF.Ln)
            lnb = tl("lnb")
            nc.scalar.activation(lnb[:], dk1[:], AF.Ln)
            wb = tl("wb")
            tt(wb[:], lna[:], lnb[:], ALU.subtract)
            nc.scalar.activation(wb[:], wb[:], AF.Exp, scale=0.5)

            om = tl("om")  # 1 - lam
            nc.vector.tensor_scalar(om[:], lam[:], -1.0, 1.0, ALU.mult, ALU.add)
            ih = tl("ih")
            nc.vector.reciprocal_approx_fast(ih[:], q["ht"])
            iw = tl("iw")
            nc.vector.reciprocal_approx_fast(iw[:], q["wd"])

            ta = tl("ta")
            tt(ta[:], om[:], wb[:], ALU.mult)       # om*wb
            tt(ta[:], ta[:], dk1[:], ALU.mult)      # om*wb*dk1
            tb = tl("tb")
            tt(tb[:], lam[:], dk[:], ALU.mult)      # lam*dk
            wc = tl("wc")
            tt(wc[:], ta[:], tb[:], ALU.add)
            tt(wc[:], wc[:], q["wd"], ALU.mult)
            tt(wc[:], wc[:], ih[:], ALU.mult)       # wc

            yb = tl("yb")
            tt(yb[:], q["yk"], q["ht"], ALU.add)
            lw = tl("lw")
            tt(lw[:], lam[:], wb[:], ALU.mult)
            ycn = tl("ycn")
            tt(ycn[:], lw[:], yb[:], ALU.mult)
            t2 = tl("t2")
            tt(t2[:], om[:], q["yk"], ALU.mult)
            tt(ycn[:], ycn[:], t2[:], ALU.add)
            ycd = tl("ycd")
            tt(ycd[:], om[:], lw[:], ALU.add)
            ycdr = tl("ycdr")
            nc.vector.reciprocal_approx_fast(ycdr[:], ycd[:])
            ycd = ycdr
            yc = tl("yc")
            tt(yc[:], ycn[:], ycd[:], ALU.mult)

            th = tl("th")
            tt(th[:], xc, q["xk"], ALU.subtract)
            tt(th[:], th[:], iw[:], ALU.mult)       # theta
            lmt = tl("lmt")
            tt(lmt[:], lam[:], th[:], ALU.subtract)  # lam - theta
            mleft = P.tile([128, CC], U8, tag="s2_ml")
            tt(mleft[:], th[:], lam[:], ALU.is_le)
            wcyc = tl("wcyc")
            tt(wcyc[:], wc[:], yc[:], ALU.mult)
            omt = tl("omt")
            nc.vector.tensor_scalar(omt[:], th[:], -1.0, 1.0, ALU.mult, ALU.add)
            wbyb = tl("wbyb")
            tt(wbyb[:], wb[:], yb[:], ALU.mult)

            numl = tl("numl")
            tt(numl[:], q["yk"], lmt[:], ALU.mult)
            t3 = tl("t3")
            tt(t3[:], wcyc[:], th[:], ALU.mult)
            tt(numl[:], numl[:], t3[:], ALU.add)
            numr = tl("numr")
            tt(numr[:], wcyc[:], omt[:], ALU.mult)
            t4 = tl("t4")
            tt(t4[:], wbyb[:], lmt[:], ALU.mult)
            tt(numr[:], numr[:], t4[:], ALU.subtract)
            num = tl("num")
            nc.vector.select(num[:], mleft[:], numl[:], numr[:])

            denl = tl("denl")
            tt(denl[:], wc[:], th[:], ALU.mult)
            tt(denl[:], denl[:], lmt[:], ALU.add)
            denr = tl("denr")
            tt(denr[:], wc[:], omt[:], ALU.mult)
            t5 = tl("t5")
            tt(t5[:], wb[:], lmt[:], ALU.mult)
            tt(denr[:], denr[:], t5[:], ALU.subtract)
            den = tl("den")
            nc.vector.select(den[:], mleft[:], denl[:], denr[:])
            denr2 = tl("denr2")
            nc.vector.reciprocal_approx_fast(denr2[:], den[:])
            den = denr2
            y = tl("y")
            tt(y[:], num[:], den[:], ALU.mult)
            # identity outside [-B, B]: inside <=> xc == x
            mins = P.tile([128, CC], U8, tag="s2_mi")
            tt(mins[:], xc, xog, ALU.is_equal)
            nc.vector.tensor_copy(zdst, xog)
            nc.vector.copy_predicated(zdst, mins[:], y[:])

        # ---------------- phase A: the z0 chain (constant spline) ----------
        z0t = [zpool.tile([128, CC], F32, tag=f"z0_{s}", name=f"z0_{s}")
               for s in range(FLOW_LENGTH + 1)]
        nc.sync.dma_start(z0t[0][:], xsp[:, :, 0])
        z1sp0 = zpool.tile([128, CC], F32, tag="z1_0")
        nc.sync.dma_start(z1sp0[:], xsp[:, :, 1])

        rowbs = []
        rb0 = rows.tile([1, ns], BF16, tag="rowb")
        nc.sync.dma_start(rb0[:], x0rowb[:])
        rowbs.append(rb0)

        for s in range(FLOW_LENGTH):
            zin = z0t[s]
            xc0 = pha.tile([128, CC], F32, tag="a_xc")
            nc.vector.tensor_scalar(xc0[:], zin[:], LEFT, RIGHT,
                                    ALU.max, ALU.min)
            cmpt = pha.tile([128, CC, 15], F32, tag="a_cmp")
            nc.vector.tensor_tensor(
                cmpt[:],
                xc0[:].unsqueeze(2).broadcast_to((128, CC, 15)),
                cCMP0[:].unsqueeze(1).broadcast_to((128, CC, 15)),
                ALU.is_ge)
            bin0 = pha.tile([128, CC], F32, tag="a_bin")
            nc.vector.tensor_reduce(bin0[:], cmpt[:], mybir.AxisListType.X,
                                    ALU.add)
            sel0 = pha.tile([128, CC, 7], F32, tag="a_sel0")
            tabv = cTAB0[:].rearrange("p (q k) -> p q k", k=16)
            for bb in range(CC // 16):
                sl = slice(16 * bb, 16 * bb + 16)
                oh0 = pha.tile([128, 16, 16], F32, tag="a_oh0", bufs=2)
                nc.vector.tensor_tensor(
                    oh0[:],
                    cIOTA[:].unsqueeze(1).broadcast_to((128, 16, 16)),
                    bin0[:, sl].unsqueeze(2).broadcast_to((128, 16, 16)),
                    ALU.is_equal)
                gm0 = pha.tile([128, 16, 7, 16], F32, tag="a_gm0")
                nc.gpsimd.tensor_tensor(
                    gm0[:],
                    tabv.unsqueeze(1).broadcast_to((128, 16, 7, 16)),
                    oh0[:].unsqueeze(2).broadcast_to((128, 16, 7, 16)),
                    ALU.mult)
                nc.vector.tensor_reduce(sel0[:, sl, :], gm0[:],
                                        mybir.AxisListType.X, ALU.add)
            q = {k: sel0[:, :, i] for i, k in enumerate(
                ["xk", "wd", "yk", "ht", "d0", "d1", "lr"])}
            emit_stage2(q, bin0[:], xc0[:], zin[:], z0t[s + 1][:], dim1=False)
            nc.sync.dma_start(z0out[s], z0t[s + 1][:])
            if s < FLOW_LENGTH - 1:
                ptr = s2psum.tile([CC, 128], F32, tag="a_tp", name="a_tp")
                nc.tensor.transpose(ptr[:], z0t[s + 1][:], cIDENT[:])
                z0tb = pha.tile([CC, 128], BF16, tag="a_z0tb")
                nc.scalar.copy(z0tb[:], ptr[:])
                rb = rows.tile([1, ns], BF16, tag="rowb")
                nc.sync.dma_start(rb[:], z0tb[:])
                rowbs.append(rb)

        # ---------------- phase B: MLP + dim-1 spline per step -------------
        zprev = z1sp0
        for s in range(FLOW_LENGTH):
            rowb = rowbs[s]
            xcs = s2pool.tile([128, CC], F32, tag="b_xc")
            nc.vector.tensor_scalar(xcs[:], zprev[:], LEFT, RIGHT,
                                    ALU.max, ALU.min)
            binacc = s2pool.tile([128, CC], F32, tag="b_bin")
            selacc = s2pool.tile([128, CC, 4], F32, tag="b_selacc")
            dacc0 = s2pool.tile([128, CC], F32, tag="b_dacc0")
            dacc1 = s2pool.tile([128, CC], F32, tag="b_dacc1")
            lacc = s2pool.tile([128, CC], F32, tag="b_lacc")

            h2tiles = {}
            for f in range(NF):
                h1b = mlp.tile([128, 2, 512], BF16, tag="h1b")
                for c in (0, 1):
                    hp1c = mpsum.tile([128, 512], F32, tag="hp1",
                                      name=f"hp1_{c}")
                    nc.tensor.matmul(hp1c[:], cW1[0:1, 128 * c:128 * c + 128],
                                     rowb[0:1, 512 * f:512 * f + 512],
                                     start=True, stop=True)
                    nc.scalar.activation(h1b[:, c, :], hp1c[:], AF.Relu,
                                         bias=cB1[:, c:c + 1])
                hp2 = mpsum.tile([128, 2, 512], F32, tag="hp2")
                for mc in (0, 1):
                    for kc in (0, 1):
                        nc.tensor.matmul(hp2[:, mc, :], cW2[:, kc, mc, :],
                                         h1b[:, kc, :],
                                         start=(kc == 0), stop=(kc == 1))
                h2b = mlp.tile([128, 2, 512], BF16, tag="h2b")
                nc.scalar.activation(h2b[:, 0, :], hp2[:, 0, :], AF.Relu,
                                     bias=cB2[:, 0:1])
                nc.scalar.activation(h2b[:, 1, :], hp2[:, 1, :], AF.Relu,
                                     bias=cB2[:, 1:2])
                h2tiles[f] = h2b

                if f % 4 != 3:
                    continue
                # ---- L3 + spline stage-1 for block b = f // 4 (2048 smp)
                b = f // 4
                pl3 = l3psum.tile([128, 1024], F32, tag="pl3")
                pl3v4 = pl3[:].rearrange("p (a b c) -> p a b c", a=2, c=64)
                pl3v = None
                for g in range(G):
                    hsrc = h2tiles[f - 3 + (g // 4)]
                    off = 128 * (g % 4)
                    for kc in (0, 1):
                        nc.tensor.matmul(pl3v4[:, g // 8, g % 8, 0:63],
                                         hsrc[:, kc, off:off + 128],
                                         cW3[:, kc, :],
                                         start=(kc == 0), stop=(kc == 1))
                # params + bias -> QP[:, :, 64:127]
                qp = blkpool.tile([128, G * 127], F32, tag="qp")
                qpv = qp[:].rearrange("p (g q) -> p g q", q=127)
                for a in (0, 1):
                    pl3a = pl3[:, 512 * a:512 * a + 512].rearrange(
                        "p (b c) -> p b c", c=64)[:, :, 0:63]
                    qpa = qp[:, 1016 * a:1016 * a + 1016].rearrange(
                        "p (b c) -> p b c", c=127)[:, :, 64:127]
                    nc.vector.scalar_tensor_tensor(
                        qpa, pl3a, 1.0,
                        cB3[:].unsqueeze(1).broadcast_to((128, 8, 63)),
                        ALU.mult, ALU.add)
                ew = blkpool.tile([128, G * 32], F32, tag="ew")
                nc.scalar.activation(
                    ew[:].rearrange("p (a b) -> p a b", b=32),
                    qpv[:, :, 64:96], AF.Exp)
                cs = blkpool.tile([128, G * 32], F32, tag="cs")
                nc.vector.tensor_tensor_scan(cs[:], cM32[:], ew[:], 0.0,
                                             ALU.mult, ALU.add)
                csv = cs[:].rearrange("p (g t k) -> p g t k", t=2, k=16)
                inv = blkpool.tile([128, G, 2], F32, tag="inv")
                nc.vector.reciprocal_approx_fast(inv[:], csv[:, :, :, 15])
                awh = blkpool.tile([128, G, 2], F32, tag="awh")
                nc.vector.tensor_scalar(awh[:], inv[:], A10C, None, ALU.mult)

                for half, nm in ((0, "w"), (1, "h")):
                    base = 32 * half  # cw at [0:16]+base? layout: cw,wid at 0..31; ch,hgt at 32..63
                    cwsl = qpv[:, :, base + 1:base + 16]
                    nc.vector.tensor_tensor(
                        cwsl, csv[:, :, half, 0:15],
                        awh[:, :, half].unsqueeze(2).broadcast_to((128, G, 15)),
                        ALU.mult)
                    nc.vector.tensor_tensor(
                        cwsl, cwsl,
                        cB16[:].unsqueeze(1).broadcast_to((128, G, 15)),
                        ALU.add)
                    nc.vector.memset(qpv[:, :, base], LEFT)
                    nc.vector.tensor_tensor(
                        qpv[:, :, base + 16:base + 31],
                        qpv[:, :, base + 1:base + 16],
                        qpv[:, :, base + 0:base + 15], ALU.subtract)
                    nc.vector.tensor_scalar(
                        qpv[:, :, base + 31], qpv[:, :, base + 15],
                        -1.0, RIGHT, ALU.mult, ALU.add)

                xcb = xcs[:, G * b:G * b + G]
                cmpb = blkpool.tile([128, G, 15], F32, tag="cmpb", bufs=3)
                nc.vector.tensor_tensor(
                    cmpb[:],
                    xcb.unsqueeze(2).broadcast_to((128, G, 15)),
                    qpv[:, :, 1:16], ALU.is_ge)
                nc.vector.tensor_reduce(binacc[:, G * b:G * b + G], cmpb[:],
                                        mybir.AxisListType.X, ALU.add)
                binb = binacc[:, G * b:G * b + G]
                oh1 = blkpool.tile([128, G, 16], F32, tag="oh1", bufs=3)
                nc.vector.tensor_tensor(
                    oh1[:],
                    cIOTA[:].unsqueeze(1).broadcast_to((128, G, 16)),
                    binb.unsqueeze(2).broadcast_to((128, G, 16)),
                    ALU.is_equal)
                gm4 = blkpool.tile([128, G, 4, 16], F32, tag="gm4")
                nc.gpsimd.tensor_tensor(
                    gm4[:],
                    qpv[:, :, 0:64].rearrange("p g (t k) -> p g t k", k=16),
                    oh1[:].unsqueeze(2).broadcast_to((128, G, 4, 16)),
                    ALU.mult)
                nc.vector.tensor_reduce(selacc[:, G * b:G * b + G, :], gm4[:],
                                        mybir.AxisListType.X, ALU.add)
                dm = blkpool.tile([128, G, 16], F32, tag="dm", bufs=3)
                nc.gpsimd.tensor_tensor(dm[:, :, 0:15], qpv[:, :, 96:111],
                                        oh1[:, :, 1:16], ALU.mult)
                nc.gpsimd.memset(dm[:, :, 15], 0.0)
                nc.vector.tensor_reduce(dacc0[:, G * b:G * b + G], dm[:],
                                        mybir.AxisListType.X, ALU.add)
                dm2 = blkpool.tile([128, G, 16], F32, tag="dm2", bufs=3)
                nc.gpsimd.tensor_tensor(dm2[:, :, 0:15], qpv[:, :, 96:111],
                                        oh1[:, :, 0:15], ALU.mult)
                nc.gpsimd.memset(dm2[:, :, 15], 0.0)
                nc.vector.tensor_reduce(dacc1[:, G * b:G * b + G], dm2[:],
                                        mybir.AxisListType.X, ALU.add)
                lm = blkpool.tile([128, G, 16], F32, tag="lm", bufs=3)
                nc.gpsimd.tensor_tensor(lm[:], qpv[:, :, 111:127], oh1[:],
                                        ALU.mult)
                nc.vector.tensor_reduce(lacc[:, G * b:G * b + G], lm[:],
                                        mybir.AxisListType.X, ALU.add)

            q = {"xk": selacc[:, :, 0], "wd": selacc[:, :, 1],
                 "yk": selacc[:, :, 2], "ht": selacc[:, :, 3],
                 "d0": dacc0[:], "d1": dacc1[:], "lr": lacc[:]}
            znext = zpool.tile([128, CC], F32, tag=f"z1_{s + 1}")
            emit_stage2(q, binacc[:], xcs[:], zprev[:], znext[:], dim1=True)
            nc.sync.dma_start(z1out[s], znext[:])
            zprev = znext


_NC_CACHE = {}


def _get_program(ns):
    if ns not in _NC_CACHE:
        _NC_CACHE[ns] = _build_program(ns)
    return _NC_CACHE[ns]


def _make_inputs(x, W1, b1, W2, b2, W3, b3, ns):
    """Host-side preprocessing -> per-core input maps."""
    bf = ml_dtypes.bfloat16
    CC = ns // 128
    G = 16
    n_cores = x.shape[0] // ns

    W3o = W3[1::2, :]          # [63, 256] (odd rows; mask m3 keeps them fully)
    b3o = b3[1::2]             # [63]
    tab0_v, cmp0_v = _np_const_spline_tables(b3[0::2])

    w1cb = W1[:, 0].astype(bf)[None, :]                    # [1, 256]
    b1t = b1.reshape(2, 128).T.astype(np.float32)          # [128, 2]
    w2tb = np.empty((128, 2, 2, 128), dtype=bf)
    for kc in range(2):
        for mc in range(2):
            w2tb[:, kc, mc, :] = W2[128 * mc:128 * mc + 128,
                                    128 * kc:128 * kc + 128].T.astype(bf)
    b2t = b2.reshape(2, 128).T.astype(np.float32)
    w3tb = np.empty((128, 2, 63), dtype=bf)
    for kc in range(2):
        w3tb[:, kc, :] = W3o[:, 128 * kc:128 * kc + 128].T.astype(bf)
    b3tab = np.broadcast_to(b3o.astype(np.float32), (128, 63)).copy()
    tab0 = np.broadcast_to(tab0_v, (128, 112)).copy()
    cmp0 = np.broadcast_to(cmp0_v, (128, 15)).copy()
    iota16 = np.broadcast_to(np.arange(16, dtype=np.float32),
                             (128, 16)).copy()
    b16 = np.broadcast_to(
        (0.01 * np.arange(1, 16) - 5.0).astype(np.float32), (128, 15)).copy()
    m32 = np.tile(np.r_[0.0, np.ones(15)].astype(np.float32), G * 2)
    m32 = np.broadcast_to(m32, (128, G * 32)).copy()
    ident = np.eye(128, dtype=np.float32)

    shared = dict(W1CB=w1cb, B1T=b1t, W2TB=w2tb, B2T=b2t, W3TB=w3tb,
                  B3TAB=b3tab, TAB0=tab0, CMP0=cmp0, IOTA16=iota16,
                  B16T=b16, M32=m32, IDENT=ident)

    in_maps = []
    for c in range(n_cores):
        xs = x[c * ns:(c + 1) * ns]                        # [ns, 2]
        xspc = xs.reshape(CC, 128, 2).transpose(1, 0, 2).copy()
        x0rowb = xs[:, 0].astype(bf)[None, :].copy()
        in_maps.append(dict(XSP=xspc.astype(np.float32), X0ROWB=x0rowb,
                            **shared))
    return in_maps


def _run(x, W1, b1, W2, b2, W3, b3, ns, trace=False):
    from concourse.bass_utils import run_bass_kernel_spmd

    n_cores = x.shape[0] // ns
    nc = _get_program(ns)
    in_maps = _make_inputs(x, W1, b1, W2, b2, W3, b3, ns)
    res = run_bass_kernel_spmd(nc, in_maps, list(range(n_cores)), trace=trace)

    n = x.shape[0]
    zs = np.empty((FLOW_LENGTH + 1, n, 2), np.float32)
    zs[0] = x
    for c in range(n_cores):
        r = res.results[c]
        lo = c * ns
        for s in range(FLOW_LENGTH):
            zs[s + 1, lo:lo + ns, 0] = r["Z0OUT"][s].T.reshape(ns)
            zs[s + 1, lo:lo + ns, 1] = r["Z1OUT"][s].T.reshape(ns)
    return zs, res


def kernel(x, W1, b1, W2, b2, W3, b3):
    x = np.ascontiguousarray(np.asarray(x, dtype=np.float32))
    zs, _ = _run(x, np.asarray(W1, np.float32), np.asarray(b1, np.float32),
                 np.asarray(W2, np.float32), np.asarray(b2, np.float32),
                 np.asarray(W3, np.float32), np.asarray(b3, np.float32),
                 NS)
    return zs



# revision 11
# speedup vs baseline: 1.5586x; 1.5586x over previous
"""Trainium2 Bass kernel for the AutoregressiveSplineDeep flow.

Computes 4 steps of a MADE-conditioned monotonic linear-rational-spline flow
over N=131072 2-d samples, data-parallel over 8 NeuronCores.

Key structure exploited: with input_dim=2 the MADE masks make the dim-0
spline parameters constants (from b3 even rows) and the dim-1 parameters a
function of the SCALAR z0 only.  So the whole MLP + softmax/cumsum pipeline
collapses to a 1-d table: we tabulate the 8 FINAL per-bin stage-2
quantities (wk, yk, lam*wk, wc*yc, wb*yb, wc, wb + cumulative widths) on a
126-point grid of z0 and evaluate them per sample with a piecewise-linear
clamped-relu basis:

  val(z0) = base + sum_g s_g * clamp01(u - g),   u = (z0+5)/h

On device per 1024-sample block: one K=1 PE matmul broadcasts the z0 row
into [128, 1024] psum (rows = u - g), scalar ACT applies relu(x + bias) and
DVE clamps to 1 -> the basis; then one [128,127] PE matmul per 128 samples
computes all interpolated table columns in fp32 psum.  Per-sample bin
selection is a 15-wide compare against the scanned cumulative widths, and
the 8 per-bin gathers are diff-form dot products (ge' . delta-table) done as
gpsimd multiplies + DVE reduces.  The rational spline is evaluated with the
wk-cancelled form  y = (yk*A + wcyc*dx) / (A + wc*dx)  (left branch, and
mirrored right branch), which needs no softplus/sigmoid/sqrt at runtime.
"""

import sys

sys.path.insert(0, "/opt/trn_rl_repo")

import numpy as np
import ml_dtypes

INPUT_DIM = 2
COUNT_BINS = 16
BOUND = 5.0
FLOW_LENGTH = 4
HIDDEN = 256
MIN_BIN = 1e-3
MIN_DERIV = 1e-3
MIN_LAMBDA = 0.025
N_FULL = 131072
N_CORES = 8
NS = N_FULL // N_CORES  # 16384 per core

LEFT, RIGHT = -BOUND, BOUND
CFREE = 1.0 - MIN_BIN * COUNT_BINS

GRID = 126                  # grid points for the z0 tabulation
H_GRID = 2 * BOUND / (GRID - 1)   # 0.08
NQ = 7                      # wk, yk, lamwk, wcyc, wbyb, wc, wb
NCOL = 15 + 16 * NQ         # 127 table columns


def _final_tables(raw):
    """raw [..., 63] -> per-bin final stage-2 quantity tables (float64).

    Returns (cw [...,17], quantities dict of [...,16])."""
    raw = raw.astype(np.float64)
    w = raw[..., 0:16]
    hh = raw[..., 16:32]
    dr = raw[..., 32:47]
    l = raw[..., 47:63]

    def smax(v):
        e = np.exp(v - v.max(-1, keepdims=True))
        return e / e.sum(-1, keepdims=True)

    widths = MIN_BIN + CFREE * smax(w)
    cw = np.concatenate([np.zeros_like(widths[..., :1]),
                         np.cumsum(widths, -1)], -1)
    cw = 2 * BOUND * cw - BOUND
    cw[..., 0] = -BOUND
    cw[..., -1] = BOUND
    wk = cw[..., 1:] - cw[..., :-1]
    heights = MIN_BIN + CFREE * smax(hh)
    ch = np.concatenate([np.zeros_like(heights[..., :1]),
                         np.cumsum(heights, -1)], -1)
    ch = 2 * BOUND * ch - BOUND
    ch[..., 0] = -BOUND
    ch[..., -1] = BOUND
    hk = ch[..., 1:] - ch[..., :-1]
    deriv = MIN_DERIV + np.log1p(np.exp(dr))
    one = np.ones_like(deriv[..., :1])
    deriv = np.concatenate([one, deriv, one], -1)           # [..., 17]
    lam = MIN_LAMBDA + (1 - 2 * MIN_LAMBDA) / (1 + np.exp(-l))
    dk = deriv[..., 0:16]
    dk1 = deriv[..., 1:17]
    wb = np.sqrt(dk / dk1)
    wc = (lam * dk + (1 - lam) * wb * dk1) * wk / hk
    yk = ch[..., 0:16]
    yb = yk + hk
    yc = ((1 - lam) * yk + lam * wb * yb) / ((1 - lam) + lam * wb)
    qs = dict(wk=wk, yk=yk, lamwk=lam * wk, wcyc=wc * yc, wbyb=wb * yb,
              wc=wc, wb=wb)
    return cw, qs


_QORDER = ["wk", "yk", "lamwk", "wcyc", "wbyb", "wc", "wb"]


def _dform(tab):
    """[..., 16] -> diff-form columns (tab0, tab1-tab0, ...)."""
    return np.concatenate([tab[..., :1], tab[..., 1:] - tab[..., :-1]], -1)


def _np_grid_tables(W1, b1, W2, b2, W3, b3):
    """Tabulate the dim-1 column table on the z0 grid; return the
    clamped-relu-basis coefficient matrix CRELU [128, NCOL] (float32 to be
    cast bf16) where rows 0,1 are base hi/lo and rows 2+g are slopes."""
    zg = -BOUND + H_GRID * np.arange(GRID)
    h1 = np.maximum(zg[:, None] * W1[None, :, 0].astype(np.float64)
                    + b1[None, :].astype(np.float64), 0.0)
    h2 = np.maximum(h1 @ W2.T.astype(np.float64)
                    + b2.astype(np.float64), 0.0)
    raw1 = h2 @ W3[1::2].T.astype(np.float64) + b3[1::2].astype(np.float64)
    cw, qs = _final_tables(raw1)                            # [GRID, ...]

    cols = np.empty((GRID, NCOL), np.float64)
    cols[:, 0:15] = cw[:, 1:16] - cw[:, 0:15]               # wf[0..14]
    for i, q in enumerate(_QORDER):
        cols[:, 15 + 16 * i:15 + 16 * (i + 1)] = _dform(qs[q])

    bf = ml_dtypes.bfloat16
    base = cols[0]
    bhi = base.astype(bf).astype(np.float64)
    blo = (base - bhi).astype(bf).astype(np.float64)
    slopes = cols[1:] - cols[:-1]                           # [GRID-1, NCOL]
    crelu = np.zeros((128, NCOL), np.float32)
    crelu[0] = bhi
    crelu[1] = blo
    crelu[2:2 + GRID - 1] = slopes                          # rows 2..126
    return crelu


def _np_const_tables(b3_even):
    """Dim-0 (constant) tables: cmp cw[1:16], wf[0:15], diff-form 7q."""
    cw, qs = _final_tables(b3_even[None, :])
    t0cmp = cw[0, 1:16]
    t0wf = cw[0, 1:16] - cw[0, 0:15]
    t0cols = np.stack([_dform(qs[q])[0] for q in _QORDER])  # [7, 16]
    return (t0cmp.astype(np.float32), t0wf.astype(np.float32),
            t0cols.astype(np.float32))


def _build_program(ns):
    import concourse.bacc as bacc
    import concourse.tile as tile
    import concourse.mybir as mybir

    F32 = mybir.dt.float32
    BF16 = mybir.dt.bfloat16

    CC = ns // 128          # 128 columns of samples-on-partitions tiles
    G = 8                   # sample-chunks (128 each) per gather block
    NB = CC // G            # 16 blocks per step

    nc = bacc.Bacc("TRN2", target_bir_lowering=False, debug=False,
                   num_devices=N_CORES)

    def din(name, shape, dt=F32):
        return nc.dram_tensor(name, list(shape), dt, kind="ExternalInput").ap()

    def dout(name, shape, dt=F32):
        return nc.dram_tensor(name, list(shape), dt,
                              kind="ExternalOutput").ap()

    t = dict(
        ns=ns, CC=CC, G=G, NB=NB,
        xsp=din("XSP", [128, CC, 2]),
        x0rowb=din("X0ROWB", [1, ns], BF16),
        crelu=din("CRELU", [128, NCOL], BF16),
        arow=din("AROW", [1, 128], BF16),
        bvec=din("BVEC", [128, 1]),
        t0cols=din("T0COLS", [128, NQ, 16]),
        t0wf=din("T0WF15", [128, 15]),
        t0cmp=din("T0CMP", [128, 15]),
        scanm=din("SCANM", [128, G * 15]),
        z0out=dout("Z0OUT", [FLOW_LENGTH, 128, CC]),
        z1out=dout("Z1OUT", [FLOW_LENGTH, 128, CC]),
    )

    with tile.TileContext(nc) as tc:
        _emit(nc, tc, t)
    nc.compile()
    return nc


def _emit(nc, tc, t):
    import concourse.mybir as mybir
    import contextlib

    F32 = mybir.dt.float32
    BF16 = mybir.dt.bfloat16
    U8 = mybir.dt.uint8
    AF = mybir.ActivationFunctionType
    ALU = mybir.AluOpType
    AX = mybir.AxisListType.X

    ns, CC, G, NB = t["ns"], t["CC"], t["G"], t["NB"]

    ctx = contextlib.ExitStack()
    with ctx:
        consts = ctx.enter_context(tc.tile_pool(name="consts", bufs=1))
        zpool = ctx.enter_context(tc.tile_pool(name="z", bufs=1))
        rows = ctx.enter_context(tc.tile_pool(name="rows", bufs=2))
        gep = ctx.enter_context(tc.tile_pool(name="ge", bufs=2))
        scr = ctx.enter_context(tc.tile_pool(name="scr", bufs=2))
        s2p = ctx.enter_context(tc.tile_pool(name="s2", bufs=2))
        rlup = ctx.enter_context(tc.tile_pool(name="rlu", bufs=3))
        bcps = ctx.enter_context(tc.tile_pool(name="bc", bufs=2,
                                              space="PSUM"))
        gaps = ctx.enter_context(tc.tile_pool(name="ga", bufs=2,
                                              space="PSUM"))

        def cload(ap, shape, dt=F32):
            tl = consts.tile(shape, dt, tag=ap.name, name=ap.name)
            nc.sync.dma_start(tl[:], ap[:])
            return tl

        cCRELU = cload(t["crelu"], [128, NCOL], BF16)
        cAROW = cload(t["arow"], [1, 128], BF16)
        cBVEC = cload(t["bvec"], [128, 1])
        cT0C = cload(t["t0cols"], [128, NQ, 16])
        cT0WF = cload(t["t0wf"], [128, 15])
        cT0CMP = cload(t["t0cmp"], [128, 15])
        cSCANM = cload(t["scanm"], [128, G * 15])

        z0t = [zpool.tile([128, CC], F32, tag=f"z0_{s}", name=f"z0_{s}")
               for s in range(FLOW_LENGTH + 1)]
        nc.sync.dma_start(z0t[0][:], t["xsp"][:, :, 0])
        z1t = [zpool.tile([128, CC], F32, tag=f"z1_{s}", name=f"z1_{s}")
               for s in range(FLOW_LENGTH + 1)]
        nc.sync.dma_start(z1t[0][:], t["xsp"][:, :, 1])

        rowtiles = [rows.tile([1, ns], BF16, tag="row", name=f"row{s}")
                    for s in range(FLOW_LENGTH)]
        nc.sync.dma_start(rowtiles[0][:], t["x0rowb"][:])

        # ---------------- stage 2: rational spline from gathered coeffs ----
        def stage2(qv, xkd, xcp5, xc, zin, zout, tagp):
            def tl(tag, dt=F32):
                return s2p.tile([128, CC], dt, tag=tagp + tag,
                                name=tagp + tag)

            q = [qv[:, :, i] for i in range(NQ)]  # wk yk lamwk wcyc wbyb wc wb
            dx = tl("dx")
            nc.vector.tensor_tensor(dx[:], xcp5, xkd, ALU.subtract)
            av = tl("A")
            nc.vector.tensor_tensor(av[:], q[2], dx[:], ALU.subtract)
            wkmdx = tl("wkmdx")
            nc.gpsimd.tensor_tensor(wkmdx[:], q[0], dx[:], ALU.subtract)
            mleft = tl("ml", U8)
            nc.vector.tensor_tensor(mleft[:], dx[:], q[2], ALU.is_le)

            t1 = tl("t1")
            nc.gpsimd.tensor_tensor(t1[:], q[1], av[:], ALU.mult)
            t2 = tl("t2")
            nc.gpsimd.tensor_tensor(t2[:], q[3], dx[:], ALU.mult)
            numl = tl("numl")
            nc.vector.tensor_tensor(numl[:], t1[:], t2[:], ALU.add)
            t3 = tl("t3")
            nc.gpsimd.tensor_tensor(t3[:], q[5], dx[:], ALU.mult)
            denl = tl("denl")
            nc.vector.tensor_tensor(denl[:], av[:], t3[:], ALU.add)
            t4 = tl("t4")
            nc.gpsimd.tensor_tensor(t4[:], q[3], wkmdx[:], ALU.mult)
            t5 = tl("t5")
            nc.gpsimd.tensor_tensor(t5[:], q[4], av[:], ALU.mult)
            numr = tl("numr")
            nc.vector.tensor_tensor(numr[:], t4[:], t5[:], ALU.subtract)
            t6 = tl("t6")
            nc.gpsimd.tensor_tensor(t6[:], q[5], wkmdx[:], ALU.mult)
            t7 = tl("t7")
            nc.gpsimd.tensor_tensor(t7[:], q[6], av[:], ALU.mult)
            denr = tl("denr")
            nc.vector.tensor_tensor(denr[:], t6[:], t7[:], ALU.subtract)

            num = tl("num")
            nc.vector.tensor_copy(num[:], numr[:])
            nc.vector.copy_predicated(num[:], mleft[:], numl[:])
            den = tl("den")
            nc.gpsimd.tensor_copy(den[:], denr[:])
            nc.vector.copy_predicated(den[:], mleft[:], denl[:])
            rden = tl("rden")
            nc.vector.reciprocal_approx_fast(rden[:], den[:])
            y = tl("y")
            nc.gpsimd.tensor_tensor(y[:], num[:], rden[:], ALU.mult)
            mins = tl("mi", U8)
            nc.vector.tensor_tensor(mins[:], xc, zin, ALU.is_equal)
            nc.gpsimd.tensor_copy(zout, zin)
            nc.vector.copy_predicated(zout, mins[:], y[:])

        # ---------------- the four flow steps ------------------------------
        for s in range(FLOW_LENGTH):
            # ======== dim 0: constant tables ========
            xc0 = s2p.tile([128, CC], F32, tag="a_xc")
            nc.vector.tensor_scalar(xc0[:], z0t[s][:], LEFT, RIGHT,
                                    ALU.max, ALU.min)
            xcp50 = s2p.tile([128, CC], F32, tag="a_xcp5")
            nc.gpsimd.tensor_scalar(xcp50[:], xc0[:], BOUND, None, ALU.add)
            ge0 = gep.tile([128, CC, 16], F32, tag="ge0")
            nc.gpsimd.memset(ge0[:, :, 0], 1.0)
            nc.vector.tensor_tensor(
                ge0[:, :, 1:16],
                xc0[:].unsqueeze(2).broadcast_to((128, CC, 15)),
                cT0CMP[:].unsqueeze(1).broadcast_to((128, CC, 15)),
                ALU.is_ge)
            sx0 = scr.tile([128, CC, 15], F32, tag="a_sx")
            nc.vector.tensor_tensor(
                sx0[:], ge0[:, :, 1:16],
                cT0WF[:].unsqueeze(1).broadcast_to((128, CC, 15)), ALU.mult)
            xkd0 = s2p.tile([128, CC], F32, tag="a_xkd")
            nc.vector.tensor_reduce(xkd0[:], sx0[:], AX, ALU.add)
            qv0 = s2p.tile([128, CC, NQ], F32, tag="a_qv")
            for b in range(NB):
                sl = slice(G * b, G * b + G)
                s7 = scr.tile([128, G, NQ, 16], F32, tag="a_s7")
                nc.gpsimd.tensor_tensor(
                    s7[:],
                    cT0C[:].unsqueeze(1).broadcast_to((128, G, NQ, 16)),
                    ge0[:, sl, :].unsqueeze(2).broadcast_to(
                        (128, G, NQ, 16)),
                    ALU.mult)
                nc.vector.tensor_reduce(qv0[:, sl, :], s7[:], AX, ALU.add)
            stage2(qv0[:], xkd0[:], xcp50[:], xc0[:], z0t[s][:],
                   z0t[s + 1][:], "a")
            nc.sync.dma_start(t["z0out"][s], z0t[s + 1][:])
            if s < FLOW_LENGTH - 1:
                z0b = s2p.tile([128, CC], BF16, tag="a_z0b")
                nc.scalar.copy(z0b[:], z0t[s + 1][:])
                z0bt = s2p.tile([128, CC], BF16, tag="a_z0bt")
                nc.sync.dma_start_transpose(z0bt[:], z0b[:])
                nc.sync.dma_start(rowtiles[s + 1][:], z0bt[:])

            # ======== dim 1: grid-interpolated tables ========
            row = rowtiles[s]
            xc1 = s2p.tile([128, CC], F32, tag="b_xc")
            nc.vector.tensor_scalar(xc1[:], z1t[s][:], LEFT, RIGHT,
                                    ALU.max, ALU.min)
            xcp51 = s2p.tile([128, CC], F32, tag="b_xcp5")
            nc.gpsimd.tensor_scalar(xcp51[:], xc1[:], BOUND, None, ALU.add)
            ge1 = gep.tile([128, CC, 16], F32, tag="ge1")
            nc.gpsimd.memset(ge1[:, :, 0], 1.0)
            xkd1 = s2p.tile([128, CC], F32, tag="b_xkd")
            qv1 = s2p.tile([128, CC, NQ], F32, tag="b_qv")

            for b in range(NB):
                sl = slice(G * b, G * b + G)
                bc = bcps.tile([128, G * 128], F32, tag="bc")
                for hh in (0, 1):
                    lo = G * 128 * b + 512 * hh
                    nc.tensor.matmul(bc[:, 512 * hh:512 * (hh + 1)],
                                     cAROW[:], row[0:1, lo:lo + 512],
                                     start=True, stop=True)
                rlu = rlup.tile([128, G * 128], BF16, tag="rlu")
                nc.scalar.activation(rlu[:], bc[:], AF.Relu, bias=cBVEC[:])
                nc.vector.tensor_scalar(rlu[:], rlu[:], 1.0, None, ALU.min)
                gp = gaps.tile([128, G, 128], F32, tag="gp")
                for g in range(G):
                    nc.tensor.matmul(gp[:, g, 0:NCOL],
                                     rlu[:, 128 * g:128 * (g + 1)],
                                     cCRELU[:], start=True, stop=True)
                wfc = scr.tile([128, G * 15], F32, tag="b_wfc")
                nc.vector.tensor_copy(
                    wfc[:].rearrange("p (g c) -> p g c", c=15),
                    gp[:, :, 0:15])
                cwp = scr.tile([128, G * 15], F32, tag="b_cwp")
                nc.vector.tensor_tensor_scan(
                    cwp[:], cSCANM[:], wfc[:], 0.0, ALU.mult, ALU.add)
                cwpv = cwp[:].rearrange("p (g c) -> p g c", c=15)
                nc.vector.tensor_tensor(
                    ge1[:, sl, 1:16],
                    xcp51[:, sl].unsqueeze(2).broadcast_to((128, G, 15)),
                    cwpv, ALU.is_ge)
                sx1 = scr.tile([128, G, 15], F32, tag="b_sx")
                nc.vector.tensor_tensor(sx1[:], ge1[:, sl, 1:16],
                                        gp[:, :, 0:15], ALU.mult)
                nc.vector.tensor_reduce(xkd1[:, sl], sx1[:], AX, ALU.add)
                s7 = scr.tile([128, G, NQ, 16], F32, tag="b_s7")
                nc.vector.tensor_tensor(
                    s7[:],
                    gp[:, :, 15:NCOL].rearrange("p g (q k) -> p g q k",
                                                k=16),
                    ge1[:, sl, :].unsqueeze(2).broadcast_to(
                        (128, G, NQ, 16)),
                    ALU.mult)
                nc.vector.tensor_reduce(qv1[:, sl, :], s7[:], AX, ALU.add)

            stage2(qv1[:], xkd1[:], xcp51[:], xc1[:], z1t[s][:],
                   z1t[s + 1][:], "b")
            nc.sync.dma_start(t["z1out"][s], z1t[s + 1][:])


_NC_CACHE = {}


def _get_program(ns):
    if ns not in _NC_CACHE:
        _NC_CACHE[ns] = _build_program(ns)
    return _NC_CACHE[ns]


def _make_inputs(x, W1, b1, W2, b2, W3, b3, ns):
    bf = ml_dtypes.bfloat16
    CC = ns // 128
    G = 8
    n_cores = x.shape[0] // ns

    crelu = _np_grid_tables(W1, b1, W2, b2, W3, b3).astype(bf)
    t0cmp_v, t0wf_v, t0cols_v = _np_const_tables(b3[0::2])

    arow = np.zeros((1, 128), np.float32)
    arow[0, 2:2 + GRID - 1] = 1.0 / H_GRID
    bvec = np.zeros((128, 1), np.float32)
    bvec[0, 0] = 1.0
    bvec[1, 0] = 1.0
    g_idx = np.arange(GRID - 1, dtype=np.float32)
    bvec[2:2 + GRID - 1, 0] = BOUND / H_GRID - g_idx

    scanm = np.tile(np.r_[0.0, np.ones(14)].astype(np.float32), G)

    shared = dict(
        CRELU=crelu,
        AROW=arow.astype(bf),
        BVEC=bvec,
        T0COLS=np.broadcast_to(t0cols_v, (128, NQ, 16)).copy(),
        T0WF15=np.broadcast_to(t0wf_v, (128, 15)).copy(),
        T0CMP=np.broadcast_to(t0cmp_v, (128, 15)).copy(),
        SCANM=np.broadcast_to(scanm, (128, G * 15)).copy(),
    )

    in_maps = []
    for c in range(n_cores):
        xs = x[c * ns:(c + 1) * ns]                        # [ns, 2]
        xspc = xs.reshape(CC, 128, 2).transpose(1, 0, 2).copy()
        x0rowb = xs[:, 0].astype(bf)[None, :].copy()
        in_maps.append(dict(XSP=xspc.astype(np.float32), X0ROWB=x0rowb,
                            **shared))
    return in_maps


def _run(x, W1, b1, W2, b2, W3, b3, ns, trace=False):
    from concourse.bass_utils import run_bass_kernel_spmd

    n_cores = x.shape[0] // ns
    nc = _get_program(ns)
    in_maps = _make_inputs(x, W1, b1, W2, b2, W3, b3, ns)
    res = run_bass_kernel_spmd(nc, in_maps, list(range(n_cores)), trace=trace)

    n = x.shape[0]
    zs = np.empty((FLOW_LENGTH + 1, n, 2), np.float32)
    zs[0] = x
    for c in range(n_cores):
        r = res.results[c]
        lo = c * ns
        for s in range(FLOW_LENGTH):
            zs[s + 1, lo:lo + ns, 0] = r["Z0OUT"][s].T.reshape(ns)
            zs[s + 1, lo:lo + ns, 1] = r["Z1OUT"][s].T.reshape(ns)
    return zs, res


def kernel(x, W1, b1, W2, b2, W3, b3):
    x = np.ascontiguousarray(np.asarray(x, dtype=np.float32))
    zs, _ = _run(x, np.asarray(W1, np.float32), np.asarray(b1, np.float32),
                 np.asarray(W2, np.float32), np.asarray(b2, np.float32),
                 np.asarray(W3, np.float32), np.asarray(b3, np.float32),
                 NS)
    return zs


# revision 19
# speedup vs baseline: 1.6773x; 1.0761x over previous
"""Trainium2 Bass kernel for the AutoregressiveSplineDeep flow.

Computes 4 steps of a MADE-conditioned monotonic linear-rational-spline flow
over N=131072 2-d samples, data-parallel over 8 NeuronCores.

Key structure exploited: with input_dim=2 the MADE masks make the dim-0
spline parameters constants (from b3 even rows) and the dim-1 parameters a
function of the SCALAR z0 only.  So the whole MLP + softmax/cumsum pipeline
collapses to a 1-d table: we tabulate the 8 FINAL per-bin stage-2
quantities (wk, yk, lam*wk, wc*yc, wb*yb, wc, wb + cumulative widths) on a
126-point grid of z0 and evaluate them per sample with a piecewise-linear
clamped-relu basis:

  val(z0) = base + sum_g s_g * clamp01(u - g),   u = (z0+5)/h

On device per 1024-sample block: one K=1 PE matmul broadcasts the z0 row
into [128, 1024] psum (rows = u - g), scalar ACT applies relu(x + bias) and
DVE clamps to 1 -> the basis; then one [128,127] PE matmul per 128 samples
computes all interpolated table columns in fp32 psum.  Per-sample bin
selection is a 15-wide compare against the scanned cumulative widths, and
the 8 per-bin gathers are diff-form dot products (ge' . delta-table) done as
gpsimd multiplies + DVE reduces.  The rational spline is evaluated with the
wk-cancelled form  y = (yk*A + wcyc*dx) / (A + wc*dx)  (left branch, and
mirrored right branch), which needs no softplus/sigmoid/sqrt at runtime.
"""

import sys

sys.path.insert(0, "/opt/trn_rl_repo")

import numpy as np
import ml_dtypes

INPUT_DIM = 2
COUNT_BINS = 16
BOUND = 5.0
FLOW_LENGTH = 4
HIDDEN = 256
MIN_BIN = 1e-3
MIN_DERIV = 1e-3
MIN_LAMBDA = 0.025
N_FULL = 131072
N_CORES = 8
NS = N_FULL // N_CORES  # 16384 per core

LEFT, RIGHT = -BOUND, BOUND
CFREE = 1.0 - MIN_BIN * COUNT_BINS

GRID = 125                  # grid points for the z0 tabulation
H_GRID = 2 * BOUND / (GRID - 1)
NQ = 7
NQ32 = 3                    # yk, wcyc, wbyb stay fp32
NQ16 = 4                    # wk, lamwk, wc, wb drained to fp16
NCOL = 15 + 16 * NQ         # 127 table columns


def _final_tables(raw):
    """raw [..., 63] -> per-bin final stage-2 quantity tables (float64).

    Returns (cw [...,17], quantities dict of [...,16])."""
    raw = raw.astype(np.float64)
    w = raw[..., 0:16]
    hh = raw[..., 16:32]
    dr = raw[..., 32:47]
    l = raw[..., 47:63]

    def smax(v):
        e = np.exp(v - v.max(-1, keepdims=True))
        return e / e.sum(-1, keepdims=True)

    widths = MIN_BIN + CFREE * smax(w)
    cw = np.concatenate([np.zeros_like(widths[..., :1]),
                         np.cumsum(widths, -1)], -1)
    cw = 2 * BOUND * cw - BOUND
    cw[..., 0] = -BOUND
    cw[..., -1] = BOUND
    wk = cw[..., 1:] - cw[..., :-1]
    heights = MIN_BIN + CFREE * smax(hh)
    ch = np.concatenate([np.zeros_like(heights[..., :1]),
                         np.cumsum(heights, -1)], -1)
    ch = 2 * BOUND * ch - BOUND
    ch[..., 0] = -BOUND
    ch[..., -1] = BOUND
    hk = ch[..., 1:] - ch[..., :-1]
    deriv = MIN_DERIV + np.log1p(np.exp(dr))
    one = np.ones_like(deriv[..., :1])
    deriv = np.concatenate([one, deriv, one], -1)           # [..., 17]
    lam = MIN_LAMBDA + (1 - 2 * MIN_LAMBDA) / (1 + np.exp(-l))
    dk = deriv[..., 0:16]
    dk1 = deriv[..., 1:17]
    wb = np.sqrt(dk / dk1)
    wc = (lam * dk + (1 - lam) * wb * dk1) * wk / hk
    yk = ch[..., 0:16]
    yb = yk + hk
    yc = ((1 - lam) * yk + lam * wb * yb) / ((1 - lam) + lam * wb)
    qs = dict(wk=wk, yk=yk, lamwk=lam * wk, wcyc=wc * yc, wbyb=wb * yb,
              wc=wc, wb=wb)
    return cw, qs


_QORDER = ["yk", "wcyc", "wbyb", "wk", "lamwk", "wc", "wb"]


def _dform(tab):
    """[..., 16] -> diff-form columns (tab0, tab1-tab0, ...)."""
    return np.concatenate([tab[..., :1], tab[..., 1:] - tab[..., :-1]], -1)


def _np_grid_tables(W1, b1, W2, b2, W3, b3):
    """Tabulate the dim-1 column table on the z0 grid; return the
    clamped-relu-basis coefficient matrix CRELU [128, NCOL] (float32 to be
    cast bf16) where rows 0,1 are base hi/lo and rows 2+g are slopes."""
    zg = -BOUND + H_GRID * np.arange(GRID)
    h1 = np.maximum(zg[:, None] * W1[None, :, 0].astype(np.float64)
                    + b1[None, :].astype(np.float64), 0.0)
    h2 = np.maximum(h1 @ W2.T.astype(np.float64)
                    + b2.astype(np.float64), 0.0)
    raw1 = h2 @ W3[1::2].T.astype(np.float64) + b3[1::2].astype(np.float64)
    cw, qs = _final_tables(raw1)                            # [GRID, ...]

    cols = np.empty((GRID, NCOL), np.float64)
    cols[:, 0:15] = cw[:, 1:16] - cw[:, 0:15]               # wf[0..14]
    for i, q in enumerate(_QORDER):
        cols[:, 15 + 16 * i:15 + 16 * (i + 1)] = _dform(qs[q])

    # fp16 relu-basis encoding:
    #   F(u) = base + s0*u + sum_{g=1..GRID-2} (s_g - s_{g-1}) * relu(u - g)
    # rows: 0,1 = base hi/lo; 2,3 = s0 hi/lo; 4.. = 2nd-diff slopes.
    f16 = np.float16
    slopes = cols[1:] - cols[:-1]                           # [GRID-1, NCOL]
    base = cols[0]
    bhi = base.astype(f16).astype(np.float64)
    blo = (base - bhi).astype(f16).astype(np.float64)
    s0 = slopes[0]
    s0hi = s0.astype(f16).astype(np.float64)
    s0lo = (s0 - s0hi).astype(f16).astype(np.float64)
    crelu = np.zeros((128, NCOL), np.float32)
    crelu[0] = bhi
    crelu[1] = blo
    crelu[2] = s0hi
    crelu[3] = s0lo
    crelu[4:4 + GRID - 2] = slopes[1:] - slopes[:-1]        # rows 4..127
    return crelu


def _np_const_tables(b3_even):
    """Dim-0 (constant) tables: cmp cw[1:16], wf[0:15], diff-form 3q+4q."""
    cw, qs = _final_tables(b3_even[None, :])
    t0cmp = cw[0, 1:16]
    t0wf = cw[0, 1:16] - cw[0, 0:15]
    t0c3 = np.stack([_dform(qs[q])[0] for q in _QORDER[:NQ32]])   # [3, 16]
    t0c4 = np.stack([_dform(qs[q])[0] for q in _QORDER[NQ32:]])   # [4, 16]
    return (t0cmp.astype(np.float32), t0wf.astype(np.float32),
            t0c3.astype(np.float32), t0c4.astype(np.float16))


def _build_program(ns):
    import concourse.bacc as bacc
    import concourse.tile as tile
    import concourse.mybir as mybir

    F32 = mybir.dt.float32
    BF16 = mybir.dt.bfloat16
    F16 = mybir.dt.float16

    CC = ns // 128          # 128 columns of samples-on-partitions tiles
    G = 8                   # sample-chunks (128 each) per gather block
    NB = CC // G            # 16 blocks per step

    nc = bacc.Bacc("TRN2", target_bir_lowering=False, debug=False,
                   num_devices=N_CORES)

    def din(name, shape, dt=F32):
        return nc.dram_tensor(name, list(shape), dt, kind="ExternalInput").ap()

    def dout(name, shape, dt=F32):
        return nc.dram_tensor(name, list(shape), dt,
                              kind="ExternalOutput").ap()

    t = dict(
        ns=ns, CC=CC, G=G, NB=NB,
        xsp=din("XSP", [128, CC, 2]),
        x0rowb=din("X0ROWB", [1, ns], BF16),
        crelu=din("CRELU", [128, NCOL], F16),
        arow=din("AROW", [1, 128], BF16),
        bvec=din("BVEC", [128, 1]),
        t0c3=din("T0C3", [128, NQ32, 16]),
        t0c4=din("T0C4", [128, NQ16, 16], F16),
        t0wf=din("T0WF15", [128, 15]),
        t0cmp=din("T0CMP", [128, 15]),
        scanm=din("SCANM", [128, G * 15]),
        z0out=dout("Z0OUT", [FLOW_LENGTH, 128, CC]),
        z1out=dout("Z1OUT", [FLOW_LENGTH, 128, CC]),
    )

    with tile.TileContext(nc) as tc:
        _emit(nc, tc, t)
    nc.compile()
    return nc


def _emit(nc, tc, t):
    import concourse.mybir as mybir
    import contextlib

    F32 = mybir.dt.float32
    BF16 = mybir.dt.bfloat16
    F16 = mybir.dt.float16
    U8 = mybir.dt.uint8
    AF = mybir.ActivationFunctionType
    ALU = mybir.AluOpType
    AX = mybir.AxisListType.X

    ns, CC, G, NB = t["ns"], t["CC"], t["G"], t["NB"]

    ctx = contextlib.ExitStack()
    with ctx:
        consts = ctx.enter_context(tc.tile_pool(name="consts", bufs=1))
        zpool = ctx.enter_context(tc.tile_pool(name="z", bufs=1))
        rows = ctx.enter_context(tc.tile_pool(name="rows", bufs=2))
        gep = ctx.enter_context(tc.tile_pool(name="ge", bufs=2))
        scr = ctx.enter_context(tc.tile_pool(name="scr", bufs=2))
        s2p = ctx.enter_context(tc.tile_pool(name="s2", bufs=2))
        rlup = ctx.enter_context(tc.tile_pool(name="rlu", bufs=3))
        bcps = ctx.enter_context(tc.tile_pool(name="bc", bufs=2,
                                              space="PSUM"))
        gaps = ctx.enter_context(tc.tile_pool(name="ga", bufs=2,
                                              space="PSUM"))

        def cload(ap, shape, dt=F32):
            tl = consts.tile(shape, dt, tag=ap.name, name=ap.name)
            nc.sync.dma_start(tl[:], ap[:])
            return tl

        cCRELU = cload(t["crelu"], [128, NCOL], F16)
        cAROW = cload(t["arow"], [1, 128], BF16)
        cBVEC = cload(t["bvec"], [128, 1])
        cT0C3 = cload(t["t0c3"], [128, NQ32, 16])
        cT0C4 = cload(t["t0c4"], [128, NQ16, 16], F16)
        cT0WF = cload(t["t0wf"], [128, 15])
        cT0CMP = cload(t["t0cmp"], [128, 15])
        cSCANM = cload(t["scanm"], [128, G * 15])

        z0t = [zpool.tile([128, CC], F32, tag=f"z0_{s}", name=f"z0_{s}")
               for s in range(FLOW_LENGTH + 1)]
        nc.sync.dma_start(z0t[0][:], t["xsp"][:, :, 0])
        z1t = [zpool.tile([128, CC], F32, tag=f"z1_{s}", name=f"z1_{s}")
               for s in range(FLOW_LENGTH + 1)]
        nc.sync.dma_start(z1t[0][:], t["xsp"][:, :, 1])

        rowtiles = [rows.tile([1, ns], BF16, tag="row", name=f"row{s}")
                    for s in range(FLOW_LENGTH)]
        nc.sync.dma_start(rowtiles[0][:], t["x0rowb"][:])

        # ---------------- stage 2: rational spline from gathered coeffs ----
        def stage2(qv, xkd, xcp5, xc, zin, zout, tagp):
            def tl(tag, dt=F32):
                return s2p.tile([128, CC], dt, tag=tagp + tag,
                                name=tagp + tag)

            # qv order: yk(0) wcyc(1) wbyb(2) wk(3) lamwk(4) wc(5) wb(6)
            q = [qv[:, :, i] for i in range(NQ)]
            dx = tl("dx")
            nc.vector.tensor_tensor(dx[:], xcp5, xkd, ALU.subtract)
            av = tl("A")
            nc.vector.tensor_tensor(av[:], q[4], dx[:], ALU.subtract)
            wkmdx = tl("wkmdx")
            nc.gpsimd.tensor_tensor(wkmdx[:], q[3], dx[:], ALU.subtract)
            mleft = tl("ml", U8)
            nc.vector.tensor_tensor(mleft[:], dx[:], q[4], ALU.is_le)

            t1 = tl("t1")
            nc.gpsimd.tensor_tensor(t1[:], q[0], av[:], ALU.mult)
            t2 = tl("t2")
            nc.gpsimd.tensor_tensor(t2[:], q[1], dx[:], ALU.mult)
            numl = tl("numl")
            nc.vector.tensor_tensor(numl[:], t1[:], t2[:], ALU.add)
            t3 = tl("t3")
            nc.gpsimd.tensor_tensor(t3[:], q[5], dx[:], ALU.mult)
            denl = tl("denl")
            nc.vector.tensor_tensor(denl[:], av[:], t3[:], ALU.add)
            t4 = tl("t4")
            nc.gpsimd.tensor_tensor(t4[:], q[1], wkmdx[:], ALU.mult)
            t5 = tl("t5")
            nc.gpsimd.tensor_tensor(t5[:], q[2], av[:], ALU.mult)
            numr = tl("numr")
            nc.vector.tensor_tensor(numr[:], t4[:], t5[:], ALU.subtract)
            t6 = tl("t6")
            nc.gpsimd.tensor_tensor(t6[:], q[5], wkmdx[:], ALU.mult)
            t7 = tl("t7")
            nc.gpsimd.tensor_tensor(t7[:], q[6], av[:], ALU.mult)
            denr = tl("denr")
            nc.vector.tensor_tensor(denr[:], t6[:], t7[:], ALU.subtract)

            num = tl("num")
            nc.vector.tensor_copy(num[:], numr[:])
            nc.vector.copy_predicated(num[:], mleft[:], numl[:])
            den = tl("den")
            nc.gpsimd.tensor_copy(den[:], denr[:])
            nc.vector.copy_predicated(den[:], mleft[:], denl[:])
            rden = tl("rden")
            nc.vector.reciprocal_approx_fast(rden[:], den[:])
            y = tl("y")
            nc.gpsimd.tensor_tensor(y[:], num[:], rden[:], ALU.mult)
            mins = tl("mi", U8)
            nc.vector.tensor_tensor(mins[:], xc, zin, ALU.is_equal)
            nc.gpsimd.tensor_copy(zout, zin)
            nc.vector.copy_predicated(zout, mins[:], y[:])

        # ---------------- the four flow steps ------------------------------
        for s in range(FLOW_LENGTH):
            # ======== dim 0: constant tables ========
            xc0 = s2p.tile([128, CC], F32, tag="a_xc")
            nc.vector.tensor_scalar(xc0[:], z0t[s][:], LEFT, RIGHT,
                                    ALU.max, ALU.min)
            xcp50 = s2p.tile([128, CC], F32, tag="a_xcp5")
            nc.vector.tensor_scalar(xcp50[:], xc0[:], BOUND, None, ALU.add)
            ge0h = gep.tile([128, CC, 16], F16, tag="ge0h")
            nc.gpsimd.memset(ge0h[:, :, 0], 1.0)
            nc.vector.tensor_tensor(
                ge0h[:, :, 1:16],
                xc0[:].unsqueeze(2).broadcast_to((128, CC, 15)),
                cT0CMP[:].unsqueeze(1).broadcast_to((128, CC, 15)),
                ALU.is_ge)
            sx0 = scr.tile([128, CC, 15], F32, tag="a_sx")
            nc.vector.tensor_tensor(
                sx0[:], ge0h[:, :, 1:16],
                cT0WF[:].unsqueeze(1).broadcast_to((128, CC, 15)), ALU.mult)
            xkd0 = s2p.tile([128, CC], F32, tag="a_xkd")
            nc.vector.tensor_reduce(xkd0[:], sx0[:], AX, ALU.add)
            qv0 = s2p.tile([128, CC, NQ], F32, tag="a_qv")
            for b in range(NB):
                sl = slice(G * b, G * b + G)
                s3 = scr.tile([128, G, NQ32, 16], F32, tag="a_s3")
                nc.gpsimd.tensor_tensor(
                    s3[:],
                    cT0C3[:].unsqueeze(1).broadcast_to((128, G, NQ32, 16)),
                    ge0h[:, sl, :].unsqueeze(2).broadcast_to(
                        (128, G, NQ32, 16)),
                    ALU.mult)
                nc.vector.tensor_reduce(qv0[:, sl, 0:NQ32], s3[:], AX,
                                        ALU.add)
                s4 = scr.tile([128, G, NQ16, 16], F16, tag="a_s4")
                nc.gpsimd.tensor_tensor(
                    s4[:],
                    cT0C4[:].unsqueeze(1).broadcast_to((128, G, NQ16, 16)),
                    ge0h[:, sl, :].unsqueeze(2).broadcast_to(
                        (128, G, NQ16, 16)),
                    ALU.mult)
                nc.vector.tensor_reduce(qv0[:, sl, NQ32:NQ], s4[:], AX,
                                        ALU.add)
            stage2(qv0[:], xkd0[:], xcp50[:], xc0[:], z0t[s][:],
                   z0t[s + 1][:], "a")
            nc.sync.dma_start(t["z0out"][s], z0t[s + 1][:])
            if s < FLOW_LENGTH - 1:
                z0b = s2p.tile([128, CC], BF16, tag="a_z0b")
                nc.scalar.copy(z0b[:], z0t[s + 1][:])
                z0bt = s2p.tile([128, CC], BF16, tag="a_z0bt")
                nc.sync.dma_start_transpose(z0bt[:], z0b[:])
                nc.sync.dma_start(rowtiles[s + 1][:], z0bt[:])

            # ======== dim 1: grid-interpolated tables ========
            row = rowtiles[s]
            xc1 = s2p.tile([128, CC], F32, tag="b_xc")
            nc.vector.tensor_scalar(xc1[:], z1t[s][:], LEFT, RIGHT,
                                    ALU.max, ALU.min)
            xcp51 = s2p.tile([128, CC], F32, tag="b_xcp5")
            nc.vector.tensor_scalar(xcp51[:], xc1[:], BOUND, None, ALU.add)
            ge1h = gep.tile([128, CC, 16], F16, tag="ge1h")
            nc.gpsimd.memset(ge1h[:, :, 0], 1.0)
            xkd1 = s2p.tile([128, CC], F32, tag="b_xkd")
            qv1 = s2p.tile([128, CC, NQ], F32, tag="b_qv")

            for b in range(NB):
                sl = slice(G * b, G * b + G)
                bc = bcps.tile([128, G * 128], F32, tag="bc")
                for hh in (0, 1):
                    lo = G * 128 * b + 512 * hh
                    nc.tensor.matmul(bc[:, 512 * hh:512 * (hh + 1)],
                                     cAROW[:], row[0:1, lo:lo + 512],
                                     start=True, stop=True)
                rlu = rlup.tile([128, G * 128], F16, tag="rlu")
                nc.scalar.activation(rlu[:], bc[:], AF.Relu, bias=cBVEC[:])
                gp = gaps.tile([128, G, 128], F32, tag="gp")
                for g in range(G):
                    nc.tensor.matmul(gp[:, g, 0:NCOL],
                                     rlu[:, 128 * g:128 * (g + 1)],
                                     cCRELU[:], start=True, stop=True)
                wfc = scr.tile([128, G * 15], F32, tag="b_wfc")
                nc.vector.tensor_copy(
                    wfc[:].rearrange("p (g c) -> p g c", c=15),
                    gp[:, :, 0:15])
                cwp = scr.tile([128, G * 15], F32, tag="b_cwp")
                nc.vector.tensor_tensor_scan(
                    cwp[:], cSCANM[:], wfc[:], 0.0, ALU.mult, ALU.add)
                cwpv = cwp[:].rearrange("p (g c) -> p g c", c=15)
                nc.vector.tensor_tensor(
                    ge1h[:, sl, 1:16],
                    xcp51[:, sl].unsqueeze(2).broadcast_to((128, G, 15)),
                    cwpv, ALU.is_ge)
                sx1 = scr.tile([128, G, 15], F32, tag="b_sx")
                nc.vector.tensor_tensor(sx1[:], ge1h[:, sl, 1:16],
                                        gp[:, :, 0:15], ALU.mult)
                nc.vector.tensor_reduce(xkd1[:, sl], sx1[:], AX, ALU.add)
                s3 = scr.tile([128, G, NQ32, 16], F32, tag="b_s3")
                nc.vector.tensor_tensor(
                    s3[:],
                    gp[:, :, 15:15 + 16 * NQ32].rearrange(
                        "p g (q k) -> p g q k", k=16),
                    ge1h[:, sl, :].unsqueeze(2).broadcast_to(
                        (128, G, NQ32, 16)),
                    ALU.mult)
                nc.vector.tensor_reduce(qv1[:, sl, 0:NQ32], s3[:], AX,
                                        ALU.add)
                d4 = scr.tile([128, G, NQ16, 16], F16, tag="b_d4")
                nc.scalar.copy(
                    d4[:],
                    gp[:, :, 15 + 16 * NQ32:NCOL].rearrange(
                        "p g (q k) -> p g q k", k=16))
                s4 = scr.tile([128, G, NQ16, 16], F16, tag="b_s4")
                nc.gpsimd.tensor_tensor(
                    s4[:], d4[:],
                    ge1h[:, sl, :].unsqueeze(2).broadcast_to(
                        (128, G, NQ16, 16)),
                    ALU.mult)
                nc.vector.tensor_reduce(qv1[:, sl, NQ32:NQ], s4[:], AX,
                                        ALU.add)

            stage2(qv1[:], xkd1[:], xcp51[:], xc1[:], z1t[s][:],
                   z1t[s + 1][:], "b")
            nc.sync.dma_start(t["z1out"][s], z1t[s + 1][:])


_NC_CACHE = {}


def _get_program(ns):
    if ns not in _NC_CACHE:
        _NC_CACHE[ns] = _build_program(ns)
    return _NC_CACHE[ns]


def _make_inputs(x, W1, b1, W2, b2, W3, b3, ns):
    bf = ml_dtypes.bfloat16
    CC = ns // 128
    G = 8
    n_cores = x.shape[0] // ns

    crelu = _np_grid_tables(W1, b1, W2, b2, W3, b3).astype(np.float16)
    t0cmp_v, t0wf_v, t0c3_v, t0c4_v = _np_const_tables(b3[0::2])

    # basis rows: 0,1 const(=1 via relu(0+1)); 2,3 = u; 4+g = relu(u - (g+1))
    arow = np.zeros((1, 128), np.float32)
    arow[0, 2:4 + GRID - 2] = 1.0 / H_GRID
    bvec = np.zeros((128, 1), np.float32)
    bvec[0, 0] = 1.0
    bvec[1, 0] = 1.0
    bvec[2, 0] = BOUND / H_GRID
    bvec[3, 0] = BOUND / H_GRID
    g_idx = np.arange(1, GRID - 1, dtype=np.float32)
    bvec[4:4 + GRID - 2, 0] = BOUND / H_GRID - g_idx

    scanm = np.tile(np.r_[0.0, np.ones(14)].astype(np.float32), G)

    shared = dict(
        CRELU=crelu,
        AROW=arow.astype(bf),
        BVEC=bvec,
        T0C3=np.broadcast_to(t0c3_v, (128, NQ32, 16)).copy(),
        T0C4=np.broadcast_to(t0c4_v, (128, NQ16, 16)).copy(),
        T0WF15=np.broadcast_to(t0wf_v, (128, 15)).copy(),
        T0CMP=np.broadcast_to(t0cmp_v, (128, 15)).copy(),
        SCANM=np.broadcast_to(scanm, (128, G * 15)).copy(),
    )

    in_maps = []
    for c in range(n_cores):
        xs = x[c * ns:(c + 1) * ns]                        # [ns, 2]
        xspc = xs.reshape(CC, 128, 2).transpose(1, 0, 2).copy()
        x0rowb = xs[:, 0].astype(bf)[None, :].copy()
        in_maps.append(dict(XSP=xspc.astype(np.float32), X0ROWB=x0rowb,
                            **shared))
    return in_maps


def _run(x, W1, b1, W2, b2, W3, b3, ns, trace=False):
    from concourse.bass_utils import run_bass_kernel_spmd

    n_cores = x.shape[0] // ns
    nc = _get_program(ns)
    in_maps = _make_inputs(x, W1, b1, W2, b2, W3, b3, ns)
    res = run_bass_kernel_spmd(nc, in_maps, list(range(n_cores)), trace=trace)

    n = x.shape[0]
    zs = np.empty((FLOW_LENGTH + 1, n, 2), np.float32)
    zs[0] = x
    for c in range(n_cores):
        r = res.results[c]
        lo = c * ns
        for s in range(FLOW_LENGTH):
            zs[s + 1, lo:lo + ns, 0] = r["Z0OUT"][s].T.reshape(ns)
            zs[s + 1, lo:lo + ns, 1] = r["Z1OUT"][s].T.reshape(ns)
    return zs, res


def kernel(x, W1, b1, W2, b2, W3, b3):
    x = np.ascontiguousarray(np.asarray(x, dtype=np.float32))
    zs, _ = _run(x, np.asarray(W1, np.float32), np.asarray(b1, np.float32),
                 np.asarray(W2, np.float32), np.asarray(b2, np.float32),
                 np.asarray(W3, np.float32), np.asarray(b3, np.float32),
                 NS)
    return zs
